# revision 26
# baseline (speedup 1.0000x reference)
"""GCN (2x GCNConv + MLP head + softmax) on 8 TRN2 NeuronCores.

Strategy (graph/data parallel, per sharding hint):
  - Nodes sharded across 8 cores (2500 rows each, padded to 2560); weights
    replicated.
  - Per conv layer: each core computes h = z @ W for its rows, pre-scales
    rows by dinv (deg^-1/2), AllGathers the scaled table (node-major, in
    two halves A=1024/B=1536 rows per core), then aggregates messages for
    edges partitioned by dst (windows of 128 dst slots) with dma_gather
    (one row-gather call per window half) + one-hot segment matmuls on
    the TensorEngine accumulating in PSUM. Segment matrices are built
    on-chip (DVE iota==slot); dinv[dst] is folded into the PSUM->SBUF
    activation scale; the bias rides the PSUM accumulation as a rank-1
    matmul (1/dinv x b). Self-loops are ordinary edges.
  - Windows 0..15 are single-pass (both halves accumulate in one PSUM
    group); the last DEFER windows run their A-half early (spilled to
    SBUF) to cover the AllGather-B latency, and combine at the end.
  - Window epilogue relus and PE-transposes into the feature-major layout
    the next matmul needs. Head: two dense layers + row softmax, pipelined
    per 512-row block.

Host-side preprocessing is limited to graph-structure work (edge sort,
degree counts, window slots, gather-index layout) + sharding.
"""

import os
from contextlib import ExitStack

import numpy as np

import concourse.bacc as bacc
import concourse.mybir as mybir
import concourse.tile as tile
from concourse.bass_utils import run_bass_kernel_spmd
from concourse.masks import make_identity

# problem shapes (hardcoded per contract)
N = 20000
E = 320000
D = 512
D_OUT = 128
NCORES = 8
RPC = 2500          # real rows per core
RPAD = 2560         # padded rows per core (20 tiles of 128)
NPAD = RPAD * NCORES
MT = RPAD // 128    # m-tiles / dst windows per core (20)
SPLIT = 1024        # AG part-A rows per core (8 m-tiles); part B = 1536
RA = SPLIT * NCORES  # table rows in region A
MA = SPLIT // 128    # m-tiles in part A
GMAX = 6           # max chunks (of 128 edges) per dma_gather call
NQ = 4              # SWDGE queues for gather rotation
DEFER = 4           # trailing windows that run A-half early + spill

MODE = os.environ.get("GNN_MODE", "bf16")

_f32 = mybir.dt.float32
_bf16 = mybir.dt.bfloat16
_i16 = mybir.dt.int16
_i32 = mybir.dt.int32


def _table_id(node):
    """Row of node in the AllGather-ed table (two part AGs: A=1024, B=1536)."""
    c, r = node // RPC, node % RPC
    h = r >= SPLIT
    return np.where(h, RA + c * (RPAD - SPLIT) + (r - SPLIT), c * SPLIT + r)


def _prepare(edge_index):
    """Edge partitioning by dst; per-core gather indices + window slots.

    Returns (per_core list of dicts, group_sizes, chunk_base, TC).
    """
    src = np.concatenate([edge_index[0], np.arange(N, dtype=np.int64)])
    dst = np.concatenate([edge_index[1], np.arange(N, dtype=np.int64)])

    order = np.argsort(dst, kind="stable")
    srcs = _table_id(src[order]).astype(np.int64)
    dsts = dst[order]

    HB = RA  # table rows in region A

    counts = np.zeros((NCORES, MT, 2), dtype=np.int64)
    core_bounds = np.searchsorted(dsts, np.arange(NCORES + 1) * RPC)
    for c in range(NCORES):
        lo, hi = core_bounds[c], core_bounds[c + 1]
        d = dsts[lo:hi] - c * RPC
        hvec = (srcs[lo:hi] >= HB).astype(np.int64)
        wb = np.searchsorted(d, np.arange(MT + 1) * 128)
        for w in range(MT):
            a, b = wb[w], wb[w + 1]
            n1 = int(hvec[a:b].sum())
            counts[c, w, 0] = (b - a) - n1
            counts[c, w, 1] = n1

    cpw = np.maximum(1, -(-counts.max(axis=0) // 128))  # [MT, 2] chunks
    TC = int(cpw.sum())
    cb = np.concatenate([[0], np.cumsum(cpw.reshape(-1))]).astype(int)
    chunk_base = cb[:-1].reshape(MT, 2)

    group_sizes = []
    for w in range(MT):
        gw = []
        for h in (0, 1):
            n = int(cpw[w, h])
            k = -(-n // GMAX)
            base, rem = divmod(n, k)
            gw.append([base + (i < rem) for i in range(k)])
        group_sizes.append(gw)

    per_core = []
    for c in range(NCORES):
        gidx = np.zeros((TC, 128), dtype=np.int16)
        wsl = np.full((TC, 128), -1.0, dtype=np.float32)   # dst slot in window
        lo, hi = core_bounds[c], core_bounds[c + 1]
        d = dsts[lo:hi] - c * RPC
        s_ids = srcs[lo:hi]
        wb = np.searchsorted(d, np.arange(MT + 1) * 128)
        for w in range(MT):
            a, b = wb[w], wb[w + 1]
            hv = s_ids[a:b] >= HB
            for h in (0, 1):
                sel = hv if h else ~hv
                sid = (s_ids[a:b][sel] - h * HB).astype(np.int16)
                slot = (d[a:b][sel] - w * 128).astype(np.float32)
                k = np.arange(sid.size)
                tg = chunk_base[w, h] + (k // 128)
                row = k % 128
                gidx[tg, row] = sid
                wsl[tg, row] = slot
        # wrapped int16 index layout, one block per gather call
        cols = []
        for w in range(MT):
            for h in (0, 1):
                t0 = int(chunk_base[w, h])
                for gsz in group_sizes[w][h]:
                    L = gidx[t0:t0 + gsz].reshape(-1)
                    cols.append(np.tile(L.reshape(-1, 16).T, (8, 1)))
                    t0 += gsz
        idx_np = np.ascontiguousarray(np.concatenate(cols, axis=1))
        per_core.append({
            "idx": idx_np,
            "wsl": np.ascontiguousarray(wsl.T),   # [128, TC]
        })
    return per_core, group_sizes, chunk_base, TC


def _build(group_sizes, chunk_base, TC):
    mdt = _bf16   # matmul-operand / gathered-table dtype
    tdt = _bf16
    trdt = _bf16

    # 2048-descriptor SWDGE rings so a 12-chunk (1536-row) gather fits
    nc = bacc.Bacc("TRN2", target_bir_lowering=False, debug=False,
                   num_devices=NCORES, num_swdge_queues=NQ,
                   dynamic_dma_scratch_size=32768)
    xT_d = nc.dram_tensor("xT", [D, RPAD], mdt, kind="ExternalInput")
    dinv_d = nc.dram_tensor("dinv", [RPAD], _f32, kind="ExternalInput")
    rdinv_d = nc.dram_tensor("rdinv", [2, RPAD], mdt, kind="ExternalInput")
    W_d = {k: nc.dram_tensor(k, [D, D], mdt, kind="ExternalInput")
           for k in ("W1", "W2", "Wf1")}
    Wf2_d = nc.dram_tensor("Wf2", [D, D_OUT], mdt, kind="ExternalInput")
    brow_d = {k: nc.dram_tensor(k, [2, D], mdt, kind="ExternalInput")
              for k in ("b1", "b2")}
    bf1_d = nc.dram_tensor("bf1", [D], _f32, kind="ExternalInput")
    bf2_d = nc.dram_tensor("bf2", [2, D_OUT], mdt, kind="ExternalInput")
    idx_d = nc.dram_tensor("idx", [128, TC * 8], _i16, kind="ExternalInput")
    wsl_d = nc.dram_tensor("wsl", [128, TC], _f32, kind="ExternalInput")
    out_d = nc.dram_tensor("out", [RPAD, D_OUT], _f32, kind="ExternalOutput")

    cc_in = [nc.dram_tensor(f"cc_in{i}", [RPAD, D], tdt, kind="Internal")
             for i in (1, 2)]
    cc_out = [nc.dram_tensor(f"cc_out{i}", [NPAD, D], tdt, kind="Internal",
                             addr_space="Shared") for i in (1, 2)]

    RG = [list(range(NCORES))]
    ACT = mybir.ActivationFunctionType
    ALU = mybir.AluOpType

    MAIN = [w for w in range(MT - DEFER)]
    DEFW = [w for w in range(MT - DEFER, MT)]

    with tile.TileContext(nc) as tc, ExitStack() as ctx:
        const = ctx.enter_context(tc.tile_pool(name="const", bufs=1))
        actT = ctx.enter_context(tc.tile_pool(name="actT", bufs=2))
        work = ctx.enter_context(tc.tile_pool(name="work", bufs=4))
        msgp = ctx.enter_context(tc.tile_pool(name="msgp", bufs=3))
        sp = ctx.enter_context(tc.tile_pool(name="sp", bufs=4))
        spillp = ctx.enter_context(tc.tile_pool(name="spillp", bufs=DEFER))
        psA = ctx.enter_context(tc.tile_pool(name="psA", bufs=2, space="PSUM"))
        psC = ctx.enter_context(tc.tile_pool(name="psC", bufs=4, space="PSUM"))
        psT = ctx.enter_context(tc.tile_pool(name="psT", bufs=2, space="PSUM"))

        # ---- constants (order roughly by first use) ----
        idx_t = const.tile([128, TC * 8], _i16)
        nc.sync.dma_start(idx_t[:], idx_d.ap())
        wsl_t = const.tile([128, TC], _f32)
        nc.sync.dma_start(wsl_t[:], wsl_d.ap())
        dinv_t = const.tile([128, MT], _f32)
        nc.sync.dma_start(dinv_t[:], dinv_d.ap().rearrange("(a p) -> p a", p=128))
        rdinv_t = const.tile([2, RPAD], mdt)
        nc.sync.dma_start(rdinv_t[:], rdinv_d.ap())
        b_row = {}
        for k in ("b1", "b2"):
            b_row[k] = const.tile([2, D], mdt, name=f"br_{k}")
            nc.sync.dma_start(b_row[k][:], brow_d[k].ap())
        w_t = {}
        w_t["W1"] = const.tile([128, 4, D], mdt, name="wt_W1")
        nc.sync.dma_start(w_t["W1"][:], W_d["W1"].ap().rearrange("(k p) n -> p k n", p=128))
        xT_t = actT.tile([128, 4, RPAD], mdt, tag="zT")
        nc.sync.dma_start(
            xT_t[:, :, 0:SPLIT],
            xT_d.ap()[:, 0:SPLIT].rearrange("(k p) m -> p k m", p=128))
        nc.sync.dma_start(
            xT_t[:, :, SPLIT:RPAD],
            xT_d.ap()[:, SPLIT:RPAD].rearrange("(k p) m -> p k m", p=128))
        for k in ("W2", "Wf1"):
            w_t[k] = const.tile([128, 4, D], mdt, name=f"wt_{k}")
            nc.sync.dma_start(w_t[k][:], W_d[k].ap().rearrange("(k p) n -> p k n", p=128))
        wf2_t = const.tile([128, 4, D_OUT], mdt)
        nc.sync.dma_start(wf2_t[:], Wf2_d.ap().rearrange("(k p) n -> p k n", p=128))
        bf1_t = const.tile([128, 4], _f32)
        nc.sync.dma_start(bf1_t[:], bf1_d.ap().rearrange("(a p) -> p a", p=128))
        bf2_row = const.tile([2, D_OUT], mdt)
        nc.sync.dma_start(bf2_row[:], bf2_d.ap())
        ones_row = const.tile([2, 128], mdt)
        nc.vector.memset(ones_row[:], 0.0)
        nc.vector.memset(ones_row[0:1, :], 1.0)
        iota_i = msgp.tile([128, GMAX, 128], _i32, tag="msg")
        nc.gpsimd.iota(iota_i[:], pattern=[[0, GMAX], [1, 128]], base=0,
                       channel_multiplier=0)
        iota_f = const.tile([128, GMAX, 128], _f32)
        nc.vector.tensor_copy(iota_f[:], iota_i[:])

        qn = [0]

        def phase_a(srcT, wt, cc, ms):
            # cc[m] = dinv * (z @ W) for this core's rows, m-tiles in ms
            for m in ms:
                ps = psA.tile([128, D], _f32, tag="psA")
                for k in range(4):
                    nc.tensor.matmul(ps[:], lhsT=srcT[:, k, m * 128:(m + 1) * 128],
                                     rhs=wt[:, k, :], start=(k == 0), stop=(k == 3))
                hs = work.tile([128, D], tdt, tag="hs")
                nc.scalar.activation(hs[:], ps[:], ACT.Copy, scale=dinv_t[:, m:m + 1])
                nc.sync.dma_start(cc.ap()[m * 128:(m + 1) * 128, :], hs[:])

        def allgather(i, h):
            if h == 0:
                nc.gpsimd.collective_compute(
                    "AllGather", mybir.AluOpType.bypass,
                    ins=[cc_in[i].ap()[0:SPLIT]],
                    outs=[cc_out[i].ap()[0:RA]], replica_groups=RG)
            else:
                nc.gpsimd.collective_compute(
                    "AllGather", mybir.AluOpType.bypass,
                    ins=[cc_in[i].ap()[SPLIT:RPAD]],
                    outs=[cc_out[i].ap()[RA:NPAD]], replica_groups=RG)

        def emit_half(w, h, ps, cc, first, last):
            """Gather + segment-matmul one (window, half). first/last control
            the PSUM accumulation group boundaries."""
            t0 = int(chunk_base[w][h])
            col0 = t0 * 8
            n = sum(group_sizes[w][h])
            src_ap = (cc.ap()[0:RA] if h == 0 else cc.ap()[RA:NPAD]).bitcast(mdt)
            done = 0
            for gsz in group_sizes[w][h]:
                nidx = gsz * 128
                msg = msgp.tile([128, GMAX, D], mdt, tag="msg")
                nc.gpsimd.dma_gather(msg[:, :gsz, :], src_ap,
                                     idx_t[:, col0:col0 + gsz * 8],
                                     nidx, nidx, D, queue_num=qn[0] % NQ)
                qn[0] += 1
                s_t = sp.tile([128, GMAX, 128], mdt, tag="S")
                nc.vector.tensor_tensor(
                    s_t[:, :gsz, :], iota_f[:, :gsz, :],
                    wsl_t[:, t0:t0 + gsz].to_broadcast([128, gsz, 128]),
                    op=ALU.is_equal)
                for t in range(gsz):
                    nc.tensor.matmul(ps[:], lhsT=s_t[:, t, :], rhs=msg[:, t, :],
                                     start=(first and done == 0),
                                     stop=(last and done == n - 1))
                    done += 1
                t0 += gsz
                col0 += gsz * 8

        def bias_mm(ps, w, b_r, first):
            # ps[d, f] += (1/dinv[d]) * b[f]; scaled by dinv in the epilogue.
            # K=2 with a zero second row (K=1 matmuls are avoided).
            nc.tensor.matmul(ps[:], lhsT=rdinv_t[:, w * 128:(w + 1) * 128],
                             rhs=b_r[:], start=first, stop=False)

        def epilogue(w, zsrc, zT_next, post):
            """relu + xbar-transpose into feature-major zT_next; zsrc is PSUM
            or SBUF pre-combined sum (already includes bias/dinv terms)."""
            zrel = work.tile([128, D], trdt, tag="zrel")
            nc.scalar.activation(zrel[:], zsrc, ACT.Relu, scale=dinv_t[:, w:w + 1])
            for q in range(4):
                nc.sync.dma_start_transpose(
                    zT_next[:, q, w * 128:(w + 1) * 128],
                    zrel[:, q * 128:(q + 1) * 128])
            if post is not None:
                post(w)

        def layer(cc, zT_next, b_r, post=None, tailpost=None):
            spills = {}
            # deferred windows: A-half now (only needs AG part A), spill
            for w in DEFW:
                ps = psC.tile([128, D], _f32, tag="psC", name=f"psd_{w}")
                emit_half(w, 0, ps, cc, first=True, last=True)
                sp_w = spillp.tile([128, D], trdt, tag="spill", name=f"spill_{w}")
                nc.scalar.activation(sp_w[:], ps[:], ACT.Copy)
                spills[w] = sp_w
            # main windows: single pass, both halves in one PSUM group
            for w in MAIN:
                ps = psC.tile([128, D], _f32, tag="psC", name=f"ps_{w}")
                bias_mm(ps, w, b_r, first=True)
                emit_half(w, 0, ps, cc, first=False, last=False)
                emit_half(w, 1, ps, cc, first=False, last=True)
                epilogue(w, ps[:], zT_next, post)
            # deferred windows: B-half + combine
            for w in DEFW:
                ps = psC.tile([128, D], _f32, tag="psC", name=f"psd2_{w}")
                bias_mm(ps, w, b_r, first=True)
                emit_half(w, 1, ps, cc, first=False, last=True)
                t1 = work.tile([128, D], _f32, tag="t1")
                nc.vector.tensor_tensor(t1[:], ps[:], spills[w][:], op=ALU.add)
                epilogue(w, t1[:], zT_next, post)
            if tailpost is not None:
                tailpost()

        # ---- layer 1 ----
        phase_a(xT_t, w_t["W1"], cc_in[0], range(MA))
        allgather(0, 0)
        phase_a(xT_t, w_t["W1"], cc_in[0], range(MA, MT))
        allgather(0, 1)

        # ---- layer 1 aggregation; layer 2 phase A + AGs fired mid-stream ----
        z1T = actT.tile([128, 4, RPAD], mdt, tag="zT")

        def l1_post(w):
            if w == MA - 1:
                phase_a(z1T, w_t["W2"], cc_in[1], range(MA))
                allgather(1, 0)
            elif w == MAIN[-1]:
                phase_a(z1T, w_t["W2"], cc_in[1], range(MA, MT - DEFER))

        def l1_tail():
            phase_a(z1T, w_t["W2"], cc_in[1], range(MT - DEFER, MT))
            allgather(1, 1)

        layer(cc_out[0], z1T, b_row["b1"], l1_post, l1_tail)

        # ---- layer 2 aggregation + head, pipelined per 512-row block ----
        z2T = actT.tile([128, 4, RPAD], mdt, tag="zT")
        z3T = actT.tile([128, 4, RPAD], mdt, tag="zT3")

        def head_block(mb):
            for q in range(4):
                ps = psA.tile([128, D], _f32, tag="psA")
                for k in range(4):
                    nc.tensor.matmul(ps[:], lhsT=w_t["Wf1"][:, k, q * 128:(q + 1) * 128],
                                     rhs=z2T[:, k, mb * 512:(mb + 1) * 512],
                                     start=(k == 0), stop=(k == 3))
                nc.scalar.activation(z3T[:, q, mb * 512:(mb + 1) * 512], ps[:],
                                     ACT.Relu, bias=bf1_t[:, q:q + 1])
            for m in range(mb * 4, (mb + 1) * 4):
                ps2 = psT.tile([128, D_OUT], _f32, tag="psT")
                nc.tensor.matmul(ps2[:], lhsT=ones_row[:],
                                 rhs=bf2_row[:], start=True, stop=False)
                for k in range(4):
                    nc.tensor.matmul(ps2[:], lhsT=z3T[:, k, m * 128:(m + 1) * 128],
                                     rhs=wf2_t[:, k, :], start=False, stop=(k == 3))
                nmx = work.tile([128, 1], _f32, tag="nmx")
                nc.vector.tensor_reduce(nmx[:], ps2[:], axis=mybir.AxisListType.X,
                                        op=ALU.max, negate=True)
                ex = work.tile([128, D_OUT], _f32, tag="ex")
                sm = work.tile([128, 1], _f32, tag="sm")
                nc.scalar.activation(ex[:], ps2[:], ACT.Exp, bias=nmx[:, :1], scale=1.0,
                                     accum_out=sm[:, :1])
                rin = work.tile([128, 1], _f32, tag="rin")
                nc.vector.reciprocal(rin[:], sm[:])
                ot = work.tile([128, D_OUT], _f32, tag="ot")
                nc.scalar.activation(ot[:], ex[:], ACT.Copy, scale=rin[:, :1])
                nc.sync.dma_start(out_d.ap()[m * 128:(m + 1) * 128, :], ot[:])

        def l2_post(w):
            if w % 4 == 3:
                head_block(w // 4)

        def l2_tail():
            head_block(MT // 4 - 1)

        layer(cc_out[1], z2T, b_row["b2"], l2_post, l2_tail)

    nc.compile()
    return nc


def _pad2(v):
    return np.stack([v, np.zeros_like(v)])


def _run(inputs, trace=False):
    x = np.asarray(inputs["x"], dtype=np.float32)
    edge_index = np.asarray(inputs["edge_index"])
    deg = np.bincount(
        np.concatenate([edge_index[1], np.arange(N, dtype=edge_index.dtype)]),
        minlength=N,
    ).astype(np.float32)
    dinv = np.zeros(N, dtype=np.float32)
    nz = deg > 0
    dinv[nz] = (1.0 / np.sqrt(deg[nz])).astype(np.float32)

    per_core, group_sizes, chunk_base, TC = _prepare(edge_index)
    nc = _build(group_sizes, chunk_base, TC)

    import ml_dtypes
    mnp = ml_dtypes.bfloat16

    in_maps = []
    for c in range(NCORES):
        xp = np.zeros((RPAD, D), dtype=np.float32)
        xp[:RPC] = x[c * RPC:(c + 1) * RPC]
        dv = np.zeros(RPAD, dtype=np.float32)
        dv[:RPC] = dinv[c * RPC:(c + 1) * RPC]
        rdv = np.zeros(RPAD, dtype=np.float32)
        nzc = dv > 0
        rdv[nzc] = 1.0 / dv[nzc]
        in_maps.append({
            "xT": np.ascontiguousarray(xp.T).astype(mnp),
            "dinv": dv,
            "rdinv": np.stack([rdv, np.zeros_like(rdv)]).astype(mnp),
            "W1": np.asarray(inputs["W1"], np.float32).astype(mnp),
            "W2": np.asarray(inputs["W2"], np.float32).astype(mnp),
            "Wf1": np.asarray(inputs["Wf1"], np.float32).astype(mnp),
            "Wf2": np.asarray(inputs["Wf2"], np.float32).astype(mnp),
            "b1": _pad2(np.asarray(inputs["b1"], np.float32)).astype(mnp),
            "b2": _pad2(np.asarray(inputs["b2"], np.float32)).astype(mnp),
            "bf1": np.asarray(inputs["bf1"], np.float32),
            "bf2": _pad2(np.asarray(inputs["bf2"], np.float32)).astype(mnp),
            "idx": per_core[c]["idx"],
            "wsl": per_core[c]["wsl"],
        })

    res = run_bass_kernel_spmd(nc, in_maps, core_ids=list(range(NCORES)),
                               trace=trace)
    out = np.concatenate([res.results[c]["out"][:RPC] for c in range(NCORES)], axis=0)
    return out, res


def kernel(**inputs):
    out, _ = _run(inputs, trace=False)
    return out


# revision 30
# speedup vs baseline: 1.8097x; 1.8097x over previous
"""GCN (2x GCNConv + MLP head + softmax) on 8 TRN2 NeuronCores.

Strategy (graph/data parallel, per sharding hint):
  - Nodes sharded across 8 cores (2500 rows each, padded to 2560); weights
    replicated.
  - Per conv layer: each core computes h = z @ W for its rows, pre-scales
    rows by dinv (deg^-1/2), AllGathers the scaled table (node-major, in
    two halves A=1024/B=1536 rows per core), then aggregates messages for
    edges partitioned by dst (windows of 128 dst slots) with dma_gather
    (one row-gather call per window half) + one-hot segment matmuls on
    the TensorEngine accumulating in PSUM. Segment matrices are built
    on-chip (DVE iota==slot); dinv[dst] is folded into the PSUM->SBUF
    activation scale; the bias rides the PSUM accumulation as a rank-1
    matmul (1/dinv x b). Self-loops are ordinary edges.
  - Windows 0..15 are single-pass (both halves accumulate in one PSUM
    group); the last DEFER windows run their A-half early (spilled to
    SBUF) to cover the AllGather-B latency, and combine at the end.
  - Window epilogue relus and PE-transposes into the feature-major layout
    the next matmul needs. Head: two dense layers + row softmax, pipelined
    per 512-row block.

Host-side preprocessing is limited to graph-structure work (edge sort,
degree counts, window slots, gather-index layout) + sharding.
"""

import os
from contextlib import ExitStack

import numpy as np

import concourse.bacc as bacc
import concourse.mybir as mybir
import concourse.tile as tile
from concourse.bass_utils import run_bass_kernel_spmd
from concourse.masks import make_identity

# problem shapes (hardcoded per contract)
N = 20000
E = 320000
D = 512
D_OUT = 128
NCORES = 8
RPC = 2500          # real rows per core
RPAD = 2560         # padded rows per core (20 tiles of 128)
NPAD = RPAD * NCORES
MT = RPAD // 128    # m-tiles / dst windows per core (20)
SPLIT = 1024        # AG part-A rows per core (8 m-tiles); part B = 1536
RA = SPLIT * NCORES  # table rows in region A
MA = SPLIT // 128    # m-tiles in part A
GMAX = 6           # max chunks (of 128 edges) per dma_gather call
NQ = 4              # SWDGE queues for gather rotation
DEFER = 4           # trailing windows that run A-half early + spill

MODE = os.environ.get("GNN_MODE", "bf16")

_f32 = mybir.dt.float32
_bf16 = mybir.dt.bfloat16
_i16 = mybir.dt.int16
_i32 = mybir.dt.int32


def _table_id(node):
    """Row of node in the AllGather-ed table (two part AGs: A=1024, B=1536)."""
    c, r = node // RPC, node % RPC
    h = r >= SPLIT
    return np.where(h, RA + c * (RPAD - SPLIT) + (r - SPLIT), c * SPLIT + r)


def _prepare(edge_index):
    """Edge partitioning by dst; per-core gather indices + window slots.

    Returns (per_core list of dicts, group_sizes, chunk_base, TC).
    """
    src = np.concatenate([edge_index[0], np.arange(N, dtype=np.int64)])
    dst = np.concatenate([edge_index[1], np.arange(N, dtype=np.int64)])

    order = np.argsort(dst, kind="stable")
    srcs = _table_id(src[order]).astype(np.int64)
    dsts = dst[order]

    HB = RA  # table rows in region A

    counts = np.zeros((NCORES, MT, 2), dtype=np.int64)
    core_bounds = np.searchsorted(dsts, np.arange(NCORES + 1) * RPC)
    for c in range(NCORES):
        lo, hi = core_bounds[c], core_bounds[c + 1]
        d = dsts[lo:hi] - c * RPC
        hvec = (srcs[lo:hi] >= HB).astype(np.int64)
        wb = np.searchsorted(d, np.arange(MT + 1) * 128)
        for w in range(MT):
            a, b = wb[w], wb[w + 1]
            n1 = int(hvec[a:b].sum())
            counts[c, w, 0] = (b - a) - n1
            counts[c, w, 1] = n1

    cpw = np.maximum(1, -(-counts.max(axis=0) // 128))  # [MT, 2] chunks
    TC = int(cpw.sum())
    cb = np.concatenate([[0], np.cumsum(cpw.reshape(-1))]).astype(int)
    chunk_base = cb[:-1].reshape(MT, 2)

    group_sizes = []
    for w in range(MT):
        gw = []
        for h in (0, 1):
            n = int(cpw[w, h])
            k = -(-n // GMAX)
            base, rem = divmod(n, k)
            gw.append([base + (i < rem) for i in range(k)])
        group_sizes.append(gw)

    per_core = []
    for c in range(NCORES):
        gidx = np.zeros((TC, 128), dtype=np.int16)
        wsl = np.full((TC, 128), -1.0, dtype=np.float32)   # dst slot in window
        lo, hi = core_bounds[c], core_bounds[c + 1]
        d = dsts[lo:hi] - c * RPC
        s_ids = srcs[lo:hi]
        wb = np.searchsorted(d, np.arange(MT + 1) * 128)
        for w in range(MT):
            a, b = wb[w], wb[w + 1]
            hv = s_ids[a:b] >= HB
            for h in (0, 1):
                sel = hv if h else ~hv
                sid = (s_ids[a:b][sel] - h * HB).astype(np.int16)
                slot = (d[a:b][sel] - w * 128).astype(np.float32)
                k = np.arange(sid.size)
                tg = chunk_base[w, h] + (k // 128)
                row = k % 128
                gidx[tg, row] = sid
                wsl[tg, row] = slot
        # wrapped int16 index layout, one block per gather call
        cols = []
        for w in range(MT):
            for h in (0, 1):
                t0 = int(chunk_base[w, h])
                for gsz in group_sizes[w][h]:
                    L = gidx[t0:t0 + gsz].reshape(-1)
                    cols.append(np.tile(L.reshape(-1, 16).T, (8, 1)))
                    t0 += gsz
        idx_np = np.ascontiguousarray(np.concatenate(cols, axis=1))
        per_core.append({
            "idx": idx_np,
            "wsl": np.ascontiguousarray(wsl.T),   # [128, TC]
        })
    return per_core, group_sizes, chunk_base, TC


def _build(group_sizes, chunk_base, TC):
    mdt = _bf16   # matmul-operand / gathered-table dtype
    tdt = _bf16
    trdt = _bf16

    nc = bacc.Bacc("TRN2", target_bir_lowering=False, debug=False,
                   num_devices=NCORES, num_swdge_queues=NQ)
    xT_d = nc.dram_tensor("xT", [D, RPAD], mdt, kind="ExternalInput")
    dinv_d = nc.dram_tensor("dinv", [RPAD], _f32, kind="ExternalInput")
    rdinv_d = nc.dram_tensor("rdinv", [2, RPAD], mdt, kind="ExternalInput")
    W_d = {k: nc.dram_tensor(k, [D, D], mdt, kind="ExternalInput")
           for k in ("W1", "W2", "Wf1")}
    Wf2_d = nc.dram_tensor("Wf2", [D, D_OUT], mdt, kind="ExternalInput")
    brow_d = {k: nc.dram_tensor(k, [2, D], mdt, kind="ExternalInput")
              for k in ("b1", "b2")}
    bf1_d = nc.dram_tensor("bf1", [D], _f32, kind="ExternalInput")
    bf2_d = nc.dram_tensor("bf2", [2, D_OUT], mdt, kind="ExternalInput")
    idx_d = nc.dram_tensor("idx", [128, TC * 8], _i16, kind="ExternalInput")
    wsl_d = nc.dram_tensor("wsl", [128, TC], _f32, kind="ExternalInput")
    out_d = nc.dram_tensor("out", [RPAD, D_OUT], _f32, kind="ExternalOutput")

    cc_in = [nc.dram_tensor(f"cc_in{i}", [RPAD, D], tdt, kind="Internal")
             for i in (1, 2)]
    cc_out = [nc.dram_tensor(f"cc_out{i}", [NPAD, D], tdt, kind="Internal",
                             addr_space="Shared") for i in (1, 2)]

    RG = [list(range(NCORES))]
    ACT = mybir.ActivationFunctionType
    ALU = mybir.AluOpType

    MAIN = [w for w in range(MT - DEFER)]
    DEFW = [w for w in range(MT - DEFER, MT)]

    with tile.TileContext(nc) as tc, ExitStack() as ctx:
        const = ctx.enter_context(tc.tile_pool(name="const", bufs=1))
        actT = ctx.enter_context(tc.tile_pool(name="actT", bufs=2))
        work = ctx.enter_context(tc.tile_pool(name="work", bufs=4))
        msgp = ctx.enter_context(tc.tile_pool(name="msgp", bufs=6))
        sp = ctx.enter_context(tc.tile_pool(name="sp", bufs=4))
        spillp = ctx.enter_context(tc.tile_pool(name="spillp", bufs=DEFER))
        psA = ctx.enter_context(tc.tile_pool(name="psA", bufs=2, space="PSUM"))
        psC = ctx.enter_context(tc.tile_pool(name="psC", bufs=4, space="PSUM"))
        psT = ctx.enter_context(tc.tile_pool(name="psT", bufs=2, space="PSUM"))

        # ---- constants (order roughly by first use) ----
        idx_t = const.tile([128, TC * 8], _i16)
        nc.sync.dma_start(idx_t[:], idx_d.ap())
        wsl_t = const.tile([128, TC], _f32)
        nc.sync.dma_start(wsl_t[:], wsl_d.ap())
        dinv_t = const.tile([128, MT], _f32)
        nc.sync.dma_start(dinv_t[:], dinv_d.ap().rearrange("(a p) -> p a", p=128))
        rdinv_t = const.tile([2, RPAD], mdt)
        nc.sync.dma_start(rdinv_t[:], rdinv_d.ap())
        b_row = {}
        for k in ("b1", "b2"):
            b_row[k] = const.tile([2, D], mdt, name=f"br_{k}")
            nc.sync.dma_start(b_row[k][:], brow_d[k].ap())
        w_t = {}
        w_t["W1"] = const.tile([128, 4, D], mdt, name="wt_W1")
        nc.sync.dma_start(w_t["W1"][:], W_d["W1"].ap().rearrange("(k p) n -> p k n", p=128))
        xT_t = actT.tile([128, 4, RPAD], mdt, tag="zT")
        nc.sync.dma_start(
            xT_t[:, :, 0:SPLIT],
            xT_d.ap()[:, 0:SPLIT].rearrange("(k p) m -> p k m", p=128))
        nc.sync.dma_start(
            xT_t[:, :, SPLIT:RPAD],
            xT_d.ap()[:, SPLIT:RPAD].rearrange("(k p) m -> p k m", p=128))
        for k in ("W2", "Wf1"):
            w_t[k] = const.tile([128, 4, D], mdt, name=f"wt_{k}")
            nc.sync.dma_start(w_t[k][:], W_d[k].ap().rearrange("(k p) n -> p k n", p=128))
        wf2_t = const.tile([128, 4, D_OUT], mdt)
        nc.sync.dma_start(wf2_t[:], Wf2_d.ap().rearrange("(k p) n -> p k n", p=128))
        bf1_t = const.tile([128, 4], _f32)
        nc.sync.dma_start(bf1_t[:], bf1_d.ap().rearrange("(a p) -> p a", p=128))
        bf2_row = const.tile([2, D_OUT], mdt)
        nc.sync.dma_start(bf2_row[:], bf2_d.ap())
        ident = const.tile([128, 128], trdt)
        make_identity(nc, ident[:])
        ones_row = const.tile([2, 128], mdt)
        nc.vector.memset(ones_row[:], 0.0)
        nc.vector.memset(ones_row[0:1, :], 1.0)
        iota_i = msgp.tile([128, GMAX, 128], _i32, tag="msg")
        nc.gpsimd.iota(iota_i[:], pattern=[[0, GMAX], [1, 128]], base=0,
                       channel_multiplier=0)
        iota_f = const.tile([128, GMAX, 128], _f32)
        nc.vector.tensor_copy(iota_f[:], iota_i[:])

        qn = [0]

        def phase_a(srcT, wt, cc, ms):
            # cc[m] = dinv * (z @ W) for this core's rows, m-tiles in ms
            for m in ms:
                ps = psA.tile([128, D], _f32, tag="psA")
                for k in range(4):
                    nc.tensor.matmul(ps[:], lhsT=srcT[:, k, m * 128:(m + 1) * 128],
                                     rhs=wt[:, k, :], start=(k == 0), stop=(k == 3))
                hs = work.tile([128, D], tdt, tag="hs")
                nc.scalar.activation(hs[:], ps[:], ACT.Copy, scale=dinv_t[:, m:m + 1])
                nc.sync.dma_start(cc.ap()[m * 128:(m + 1) * 128, :], hs[:])

        def allgather(i, h):
            if h == 0:
                nc.gpsimd.collective_compute(
                    "AllGather", mybir.AluOpType.bypass,
                    ins=[cc_in[i].ap()[0:SPLIT]],
                    outs=[cc_out[i].ap()[0:RA]], replica_groups=RG)
            else:
                nc.gpsimd.collective_compute(
                    "AllGather", mybir.AluOpType.bypass,
                    ins=[cc_in[i].ap()[SPLIT:RPAD]],
                    outs=[cc_out[i].ap()[RA:NPAD]], replica_groups=RG)

        def emit_half(w, h, ps, cc, first, last):
            """Gather + segment-matmul one (window, half). first/last control
            the PSUM accumulation group boundaries."""
            t0 = int(chunk_base[w][h])
            col0 = t0 * 8
            n = sum(group_sizes[w][h])
            src_ap = (cc.ap()[0:RA] if h == 0 else cc.ap()[RA:NPAD]).bitcast(mdt)
            done = 0
            for gsz in group_sizes[w][h]:
                nidx = gsz * 128
                msg = msgp.tile([128, GMAX, D], mdt, tag="msg")
                nc.gpsimd.dma_gather(msg[:, :gsz, :], src_ap,
                                     idx_t[:, col0:col0 + gsz * 8],
                                     nidx, nidx, D, queue_num=qn[0] % NQ)
                qn[0] += 1
                s_t = sp.tile([128, GMAX, 128], mdt, tag="S")
                nc.vector.tensor_tensor(
                    s_t[:, :gsz, :], iota_f[:, :gsz, :],
                    wsl_t[:, t0:t0 + gsz].to_broadcast([128, gsz, 128]),
                    op=ALU.is_equal)
                for t in range(gsz):
                    nc.tensor.matmul(ps[:], lhsT=s_t[:, t, :], rhs=msg[:, t, :],
                                     start=(first and done == 0),
                                     stop=(last and done == n - 1))
                    done += 1
                t0 += gsz
                col0 += gsz * 8

        def bias_mm(ps, w, b_r, first):
            # ps[d, f] += (1/dinv[d]) * b[f]; scaled by dinv in the epilogue.
            # K=2 with a zero second row (K=1 matmuls are avoided).
            nc.tensor.matmul(ps[:], lhsT=rdinv_t[:, w * 128:(w + 1) * 128],
                             rhs=b_r[:], start=first, stop=False)

        def epilogue(w, zsrc, zT_next, post):
            """relu + transpose into feature-major zT_next; zsrc is PSUM or
            SBUF pre-combined sum (already includes bias/dinv terms)."""
            zrel = work.tile([128, D], trdt, tag="zrel")
            nc.scalar.activation(zrel[:], zsrc, ACT.Relu, scale=dinv_t[:, w:w + 1])
            for q in range(4):
                pt = psT.tile([128, 128], trdt, tag="psT")
                nc.tensor.transpose(pt[:], zrel[:, q * 128:(q + 1) * 128], ident[:])
                nc.vector.tensor_copy(zT_next[:, q, w * 128:(w + 1) * 128], pt[:])
            if post is not None:
                post(w)

        def layer(cc, zT_next, b_r, post=None, tailpost=None):
            spills = {}
            # deferred windows: A-half now (only needs AG part A), spill
            for w in DEFW:
                ps = psC.tile([128, D], _f32, tag="psC", name=f"psd_{w}")
                emit_half(w, 0, ps, cc, first=True, last=True)
                sp_w = spillp.tile([128, D], trdt, tag="spill", name=f"spill_{w}")
                nc.scalar.activation(sp_w[:], ps[:], ACT.Copy)
                spills[w] = sp_w
            # main windows: single pass, both halves in one PSUM group
            for w in MAIN:
                ps = psC.tile([128, D], _f32, tag="psC", name=f"ps_{w}")
                bias_mm(ps, w, b_r, first=True)
                emit_half(w, 0, ps, cc, first=False, last=False)
                emit_half(w, 1, ps, cc, first=False, last=True)
                epilogue(w, ps[:], zT_next, post)
            # deferred windows: B-half + combine
            for w in DEFW:
                ps = psC.tile([128, D], _f32, tag="psC", name=f"psd2_{w}")
                bias_mm(ps, w, b_r, first=True)
                emit_half(w, 1, ps, cc, first=False, last=True)
                t1 = work.tile([128, D], _f32, tag="t1")
                nc.vector.tensor_tensor(t1[:], ps[:], spills[w][:], op=ALU.add)
                epilogue(w, t1[:], zT_next, post)
            if tailpost is not None:
                tailpost()

        # ---- layer 1 ----
        phase_a(xT_t, w_t["W1"], cc_in[0], range(MA))
        allgather(0, 0)
        phase_a(xT_t, w_t["W1"], cc_in[0], range(MA, MT))
        allgather(0, 1)

        # ---- layer 1 aggregation; layer 2 phase A + AGs fired mid-stream ----
        z1T = actT.tile([128, 4, RPAD], mdt, tag="zT")

        def l1_post(w):
            if w == MA - 1:
                phase_a(z1T, w_t["W2"], cc_in[1], range(MA))
                allgather(1, 0)
            elif w == MAIN[-1]:
                phase_a(z1T, w_t["W2"], cc_in[1], range(MA, MT - DEFER))

        def l1_tail():
            phase_a(z1T, w_t["W2"], cc_in[1], range(MT - DEFER, MT))
            allgather(1, 1)

        layer(cc_out[0], z1T, b_row["b1"], l1_post, l1_tail)

        # ---- layer 2 aggregation + head, pipelined per 512-row block ----
        z2T = actT.tile([128, 4, RPAD], mdt, tag="zT")
        z3T = actT.tile([128, 4, RPAD], mdt, tag="zT3")

        def head_block(mb):
            for q in range(4):
                ps = psA.tile([128, D], _f32, tag="psA")
                for k in range(4):
                    nc.tensor.matmul(ps[:], lhsT=w_t["Wf1"][:, k, q * 128:(q + 1) * 128],
                                     rhs=z2T[:, k, mb * 512:(mb + 1) * 512],
                                     start=(k == 0), stop=(k == 3))
                nc.scalar.activation(z3T[:, q, mb * 512:(mb + 1) * 512], ps[:],
                                     ACT.Relu, bias=bf1_t[:, q:q + 1])
            for m in range(mb * 4, (mb + 1) * 4):
                ps2 = psT.tile([128, D_OUT], _f32, tag="psT")
                nc.tensor.matmul(ps2[:], lhsT=ones_row[:],
                                 rhs=bf2_row[:], start=True, stop=False)
                for k in range(4):
                    nc.tensor.matmul(ps2[:], lhsT=z3T[:, k, m * 128:(m + 1) * 128],
                                     rhs=wf2_t[:, k, :], start=False, stop=(k == 3))
                nmx = work.tile([128, 1], _f32, tag="nmx")
                nc.vector.tensor_reduce(nmx[:], ps2[:], axis=mybir.AxisListType.X,
                                        op=ALU.max, negate=True)
                ex = work.tile([128, D_OUT], _f32, tag="ex")
                sm = work.tile([128, 1], _f32, tag="sm")
                nc.scalar.activation(ex[:], ps2[:], ACT.Exp, bias=nmx[:, :1], scale=1.0,
                                     accum_out=sm[:, :1])
                rin = work.tile([128, 1], _f32, tag="rin")
                nc.vector.reciprocal(rin[:], sm[:])
                ot = work.tile([128, D_OUT], _f32, tag="ot")
                nc.scalar.activation(ot[:], ex[:], ACT.Copy, scale=rin[:, :1])
                nc.sync.dma_start(out_d.ap()[m * 128:(m + 1) * 128, :], ot[:])

        def l2_post(w):
            if w % 4 == 3:
                head_block(w // 4)

        def l2_tail():
            head_block(MT // 4 - 1)

        layer(cc_out[1], z2T, b_row["b2"], l2_post, l2_tail)

    nc.compile()
    return nc


def _pad2(v):
    return np.stack([v, np.zeros_like(v)])


def _run(inputs, trace=False):
    x = np.asarray(inputs["x"], dtype=np.float32)
    edge_index = np.asarray(inputs["edge_index"])
    deg = np.bincount(
        np.concatenate([edge_index[1], np.arange(N, dtype=edge_index.dtype)]),
        minlength=N,
    ).astype(np.float32)
    dinv = np.zeros(N, dtype=np.float32)
    nz = deg > 0
    dinv[nz] = (1.0 / np.sqrt(deg[nz])).astype(np.float32)

    per_core, group_sizes, chunk_base, TC = _prepare(edge_index)
    nc = _build(group_sizes, chunk_base, TC)

    import ml_dtypes
    mnp = ml_dtypes.bfloat16

    in_maps = []
    for c in range(NCORES):
        xp = np.zeros((RPAD, D), dtype=np.float32)
        xp[:RPC] = x[c * RPC:(c + 1) * RPC]
        dv = np.zeros(RPAD, dtype=np.float32)
        dv[:RPC] = dinv[c * RPC:(c + 1) * RPC]
        rdv = np.zeros(RPAD, dtype=np.float32)
        nzc = dv > 0
        rdv[nzc] = 1.0 / dv[nzc]
        in_maps.append({
            "xT": np.ascontiguousarray(xp.T).astype(mnp),
            "dinv": dv,
            "rdinv": np.stack([rdv, np.zeros_like(rdv)]).astype(mnp),
            "W1": np.asarray(inputs["W1"], np.float32).astype(mnp),
            "W2": np.asarray(inputs["W2"], np.float32).astype(mnp),
            "Wf1": np.asarray(inputs["Wf1"], np.float32).astype(mnp),
            "Wf2": np.asarray(inputs["Wf2"], np.float32).astype(mnp),
            "b1": _pad2(np.asarray(inputs["b1"], np.float32)).astype(mnp),
            "b2": _pad2(np.asarray(inputs["b2"], np.float32)).astype(mnp),
            "bf1": np.asarray(inputs["bf1"], np.float32),
            "bf2": _pad2(np.asarray(inputs["bf2"], np.float32)).astype(mnp),
            "idx": per_core[c]["idx"],
            "wsl": per_core[c]["wsl"],
        })

    res = run_bass_kernel_spmd(nc, in_maps, core_ids=list(range(NCORES)),
                               trace=trace)
    out = np.concatenate([res.results[c]["out"][:RPC] for c in range(NCORES)], axis=0)
    return out, res


def kernel(**inputs):
    out, _ = _run(inputs, trace=False)
    return out
